# revision 26
# baseline (speedup 1.0000x reference)
"""Multi-head attention kernel for Trainium2, data-parallel over 8 NeuronCores.

Problem: B=16, N=1024, D=768, H=12 heads (hd=64), fp32 I/O.
  qkv = x @ w_qkv + b_qkv ; attention ; out = attn_out @ w_proj + b_proj

Sharding: batch data-parallel — core c handles batches [2c, 2c+2); weights
replicated. Inside each core, the two batches are processed sequentially.

Layout strategy (bf16 matmuls at 1 cyc/row; PE held continuously busy so it
ramps to the 2.4 GHz p-state):
  - host pre-transposes x to xT [768, T] so the in-feature contraction has
    features on partitions for both operands.
  - Q^T, K^T computed feature-major [768, N]: lhsT = w_qkv cols, rhs = xT.
    A 128-row feature tile holds a PAIR of heads (2x64) -> scores matmuls
    for the two heads run packed via tile_position rows (K=64).
  - V computed token-major [N, 768], stored bf16 with a ones column appended
    per head (v_ext [128, 65]).
  - scores^T tiles [128 j, 512 q] per head -> one ACT exp op [128, 1024]
    covers both heads of a pair (softmax scale folded into exp's scale).
  - U^T = sum_j exp * v_ext accumulates in PSUM [65, 512]; row 64 is the
    softmax denominator (ones column).
  - normalization: reciprocal_approx_fast on the two [1,512] denominator
    rows straight from PSUM -> dinv2 [2,512]; a rank-2 f32r outer-product
    matmul broadcasts both rows into rb [128,512] PSUM; one tensor_mul
    (staged U x rb) writes the normalized attn output into uT.
  - proj: lhsT = uT tile, rhs = w_proj; + b_proj via broadcast add.
    Output lands token-major [T, 768] == final layout.
  - scores are emitted one key-tile ahead of attn@V so exp latency is
    hidden; QKV of the next batch / proj of the previous batch are
    interleaved into the attention blocks as TensorE filler.
"""

import contextlib
import ctypes
import os
import sys
import types

import numpy as np

# ---------------------------------------------------------------------------
# NTFF profiling shim: bass_utils's trace path imports
# antenv.axon_hooks.get_axon_ntff_profile_hook, which this container's antenv
# lacks. Register a ctypes-based equivalent so BASS_TRACE=1 works. Harmless
# if tracing is never requested.
# ---------------------------------------------------------------------------


def _install_ntff_shim():
    if "antenv.axon_hooks" in sys.modules:
        return
    so_path = "/opt/axon/libaxon_pjrt.so"
    hook = None
    try:
        lib = ctypes.CDLL(so_path)
        if hasattr(lib, "axon_start_nrt_profile"):
            lib.axon_start_nrt_profile.argtypes = [
                ctypes.POINTER(ctypes.c_int64),
                ctypes.c_size_t,
            ]
            lib.axon_start_nrt_profile.restype = ctypes.c_int64
            lib.axon_stop_nrt_profile.argtypes = [ctypes.c_char_p]
            lib.axon_stop_nrt_profile.restype = ctypes.c_int64

            @contextlib.contextmanager
            def _hook(output_dir, device_ids):
                import jax

                jax.devices()
                if device_ids:
                    ids = (ctypes.c_int64 * len(device_ids))(*device_ids)
                    rc = lib.axon_start_nrt_profile(ids, len(device_ids))
                else:
                    rc = lib.axon_start_nrt_profile(None, 0)
                if rc != 0:
                    raise RuntimeError(f"axon_start_nrt_profile rc={rc}")
                try:
                    yield
                finally:
                    n = lib.axon_stop_nrt_profile(str(output_dir).encode())
                    print(f"ntff profile: {n} file(s) in {output_dir}",
                          file=sys.stderr)

            hook = _hook
    except OSError:
        pass
    mod = types.ModuleType("antenv.axon_hooks")
    mod.get_axon_ntff_profile_hook = lambda: hook
    mod.set_axon_ntff_profile_hook = lambda h: None
    sys.modules["antenv.axon_hooks"] = mod


_install_ntff_shim()

import concourse.bacc as bacc  # noqa: E402
import concourse.mybir as mybir  # noqa: E402
import concourse.tile as tile  # noqa: E402
from concourse.bass_utils import run_bass_kernel_spmd  # noqa: E402

F32 = mybir.dt.float32
F32R = mybir.dt.float32r
BF16 = mybir.dt.bfloat16
AF = mybir.ActivationFunctionType

# Problem constants (per core)
NB = 2        # batches per core
TN = 1024     # tokens per batch
T = NB * TN   # tokens per core
D = 768
H = 12
HD = 64
D3 = 3 * D
KT = D // 128          # 6 contraction tiles
NPAIR = H // 2         # 6 head pairs
NJT = TN // 128        # 8 key tiles per batch
SCALE = HD ** -0.5


def build():
    nc = bacc.Bacc(None)
    xT_d = nc.declare_dram_parameter("xT", [D, T], BF16, isOutput=False)
    wqkv_d = nc.declare_dram_parameter("wqkv", [D, D3], BF16, isOutput=False)
    wproj_d = nc.declare_dram_parameter("wproj", [D, D], BF16, isOutput=False)
    bqk_d = nc.declare_dram_parameter("bqk", [128, 12], F32, isOutput=False)
    bv_d = nc.declare_dram_parameter("bv", [1, D], BF16, isOutput=False)
    bproj_d = nc.declare_dram_parameter("bproj", [1, D], BF16, isOutput=False)
    out_d = nc.declare_dram_parameter("out", [T, D], F32, isOutput=True)

    with tile.TileContext(nc) as tc:
        with (
            nc.allow_low_precision(reason="bf16 attention pipeline"),
            tc.tile_pool(name="const", bufs=1) as cpool,
            tc.tile_pool(name="xu", bufs=2) as xupool,
            tc.tile_pool(name="qk", bufs=2) as qkpool,
            tc.tile_pool(name="vsb", bufs=2) as vpool,
            tc.tile_pool(name="esb", bufs=4) as epool,
            tc.tile_pool(name="stg", bufs=3) as spool,
            tc.tile_pool(name="dnv", bufs=3) as dpool,
            tc.tile_pool(name="rbp", bufs=4) as rbpool,
            tc.tile_pool(name="osb", bufs=3) as opool,
            tc.tile_pool(name="psS", bufs=2, space="PSUM") as psS,
            tc.tile_pool(name="psU", bufs=2, space="PSUM") as psU,
            tc.tile_pool(name="psQ", bufs=2, space="PSUM") as psQ,
        ):
            def dma_striped(dst, src, stripes):
                # split one logical transfer into partition stripes — each
                # dma_start lands on its own hardware queue, so this divides
                # the per-queue descriptor serialization
                step = 128 // stripes
                for s in range(stripes):
                    nc.sync.dma_start(
                        dst[s * step : (s + 1) * step], src[s * step : (s + 1) * step]
                    )

            # ---- constants / weights (resident) ----
            wqkv = cpool.tile([128, KT, D3], BF16, tag="wqkv")
            wqkv_src = wqkv_d.ap().rearrange("(ko p) n -> p ko n", p=128)
            # k-sliced so the first QK k-accumulation paces with the DMAs
            for k in range(KT):
                dma_striped(
                    wqkv[:, k : k + 1, :], wqkv_src[:, k : k + 1, :], 2
                )
            wproj = cpool.tile([128, KT, D], BF16, tag="wproj")
            wproj_src = wproj_d.ap().rearrange("(ko p) n -> p ko n", p=128)
            for k in range(KT):
                nc.sync.dma_start(
                    wproj[:, k : k + 1, :], wproj_src[:, k : k + 1, :]
                )
            bqk = cpool.tile([128, 12], F32, tag="bqk")
            nc.sync.dma_start(bqk[:], bqk_d.ap())
            bv1 = cpool.tile([1, D], BF16, tag="bv1")
            nc.sync.dma_start(bv1[:], bv_d.ap())
            bvb = cpool.tile([128, D], BF16, tag="bvb")
            nc.gpsimd.partition_broadcast(bvb[:], bv1[:])
            bproj1 = cpool.tile([1, D], BF16, tag="bproj1")
            nc.sync.dma_start(bproj1[:], bproj_d.ap())
            bprojb = cpool.tile([128, D], BF16, tag="bprojb")
            nc.gpsimd.partition_broadcast(bprojb[:], bproj1[:])


            # --- per-batch contexts -------------------------------------
            class Batch:
                pass

            bats = []
            for b in range(NB):
                B_ = Batch()
                B_.tok0 = b * TN
                B_.qT = qkpool.tile([128, NPAIR, TN], BF16, tag="qT",
                                    name=f"qT{b}")
                B_.kT = qkpool.tile([128, NPAIR, TN], BF16, tag="kT",
                                    name=f"kT{b}")
                B_.vsb = vpool.tile([128, NJT, H, HD + 1], BF16, tag="v",
                                    name=f"v{b}")
                B_.xT = xupool.tile([128, KT, TN], BF16, tag="x",
                                    name=f"xT{b}")
                B_.uT = xupool.tile([128, KT, TN], BF16, tag="u",
                                    name=f"uT{b}")
                bats.append(B_)

            def emit_x_dma(B_):
                # ones column of v_ext via memset — as a DMA this fragments
                # into ~12k 2-byte descriptors and stalls startup ~20us
                nc.vector.memset(B_.vsb[:, :, :, HD : HD + 1], 1.0)
                xT_src = xT_d.ap().rearrange("(ko p) n -> p ko n", p=128)[
                    :, :, B_.tok0 : B_.tok0 + TN
                ]
                for k in range(KT):
                    dma_striped(
                        B_.xT[:, k : k + 1, :], xT_src[:, k : k + 1, :], 2
                    )

            def emit_qk_unit(B_, m, ih):
                # one [128 feat, 512 tok] Q^T or K^T tile (m<6: Q, else K)
                dst = B_.qT if m < 6 else B_.kT
                hp = m % 6
                ps = psQ.tile([128, 512], F32, tag="ps")
                for k in range(KT):
                    nc.tensor.matmul(
                        ps[:],
                        wqkv[:, k, m * 128 : (m + 1) * 128],
                        B_.xT[:, k, ih * 512 : (ih + 1) * 512],
                        start=(k == 0),
                        stop=(k == KT - 1),
                    )
                nc.vector.tensor_scalar_add(
                    dst[:, hp, ih * 512 : (ih + 1) * 512],
                    ps[:],
                    bqk[:, m : m + 1],
                )

            def emit_v_unit(B_, t, nh):
                # one [128 tok, 384 feat] V tile into the v_ext slots
                ps = psQ.tile([128, 384], F32, tag="ps")
                for k in range(KT):
                    nc.tensor.matmul(
                        ps[:],
                        B_.xT[:, k, t * 128 : (t + 1) * 128],
                        wqkv[:, k, 2 * D + nh * 384 : 2 * D + (nh + 1) * 384],
                        start=(k == 0),
                        stop=(k == KT - 1),
                    )
                nc.vector.tensor_add(
                    B_.vsb[:, t, nh * 6 : (nh + 1) * 6, 0:HD],
                    ps[:],
                    bvb[:, nh * 384 : (nh + 1) * 384],
                )

            def emit_proj_unit(B_, t, nh):
                ps = psQ.tile([128, 384], F32, tag="ps")
                for k in range(KT):
                    nc.tensor.matmul(
                        ps[:],
                        B_.uT[:, k, t * 128 : (t + 1) * 128],
                        wproj[:, k, nh * 384 : (nh + 1) * 384],
                        start=(k == 0),
                        stop=(k == KT - 1),
                    )
                ot = opool.tile([128, 384], F32, tag="o")
                nc.vector.tensor_add(
                    ot[:], ps[:], bprojb[:, nh * 384 : (nh + 1) * 384]
                )
                nc.sync.dma_start(
                    out_d.ap()[
                        B_.tok0 + t * 128 : B_.tok0 + (t + 1) * 128,
                        nh * 384 : (nh + 1) * 384,
                    ],
                    ot[:],
                )

            # ---- filler queue: TensorE work interleaved into attention ---
            fill_queue = []

            def pop_fill(n):
                for _ in range(n):
                    if fill_queue:
                        fill_queue.pop(0)()

            def emit_norm(pending):
                """Broadcast 1/denom (GpSimd, otherwise idle; full-tile dst
                from partition 0 — the only pattern the library op supports)
                and write the normalized pair block into uT."""
                B_, hp, ih, ust, dinv2 = pending
                for h in range(2):
                    rb = rbpool.tile([128, 512], F32, tag="rb",
                                     name=f"rb{h}")
                    nc.gpsimd.partition_broadcast(
                        rb[:], dinv2[0:1, h * 512 : (h + 1) * 512]
                    )
                    nc.vector.tensor_mul(
                        B_.uT[
                            h * 64 : (h + 1) * 64,
                            hp,
                            ih * 512 : (ih + 1) * 512,
                        ],
                        ust[h * 64 : (h + 1) * 64, :],
                        rb[h * 64 : (h + 1) * 64, :],
                    )

            def emit_attn_block(B_, hp, ih, pending):
                """One (head-pair, query-half) attention block: 8 key tiles of
                scores+exp+attnV, scores one jt ahead of attnV."""
                i0 = ih * 512
                pu = [
                    psU.tile([HD + 1, 512], F32, tag="pu", name=f"pu{h}")
                    for h in range(2)
                ]
                prev_e = None

                def attnv(e, jt):
                    for h in range(2):
                        nc.tensor.matmul(
                            pu[h][:],
                            B_.vsb[:, jt, 2 * hp + h, :],
                            e[:, h * 512 : (h + 1) * 512],
                            start=(jt == 0),
                            stop=(jt == NJT - 1),
                        )

                for jt in range(NJT):
                    ps = psS.tile([128, 1024], F32, tag="s")
                    for h in range(2):
                        nc.tensor.matmul(
                            ps[:, h * 512 : (h + 1) * 512],
                            B_.kT[
                                h * 64 : (h + 1) * 64,
                                hp,
                                jt * 128 : (jt + 1) * 128,
                            ],
                            B_.qT[h * 64 : (h + 1) * 64, hp, i0 : i0 + 512],
                        )
                    e = epool.tile([128, 1024], BF16, tag="e")
                    nc.scalar.activation(e[:], ps[:], AF.Exp, scale=SCALE)
                    if prev_e is not None:
                        attnv(prev_e, jt - 1)
                    if jt == 1 and pending is not None:
                        emit_norm(pending)
                        pending = None
                    if jt in (0, 2, 4, 6):
                        pop_fill(1)
                    prev_e = e
                attnv(prev_e, NJT - 1)
                if pending is not None:
                    emit_norm(pending)

                # evacuate: denominator rows to SBUF staging, then one
                # reciprocal over both; data rows -> ust staging. pu slots
                # release after the copies.
                dtmp = dpool.tile([1, 1024], F32, tag="dtmp", bufs=2)
                dinv2 = dpool.tile([1, 1024], F32, tag="dinv", bufs=2)
                ust = spool.tile([128, 512], F32, tag="ust")
                for h in range(2):
                    nc.vector.tensor_copy(
                        dtmp[0:1, h * 512 : (h + 1) * 512],
                        pu[h][HD : HD + 1, :],
                    )
                    nc.vector.tensor_copy(
                        ust[h * 64 : (h + 1) * 64, :], pu[h][0:HD, :]
                    )
                nc.vector.reciprocal_approx_fast(dinv2[0:1, :], dtmp[0:1, :])
                return (B_, hp, ih, ust, dinv2)

            # ================= global schedule =================
            b0, b1 = bats
            emit_x_dma(b0)
            # b0 prologue: V (nh=0 covers head pairs 0..2) + Q/K pairs 0,1
            for m in (0, 6):
                for ih in range(2):
                    emit_qk_unit(b0, m, ih)
            for t in range(NJT):
                emit_v_unit(b0, t, 0)
            for m in (1, 7):
                for ih in range(2):
                    emit_qk_unit(b0, m, ih)
            emit_x_dma(b1)

            # filler during b0 attention, ordered by consumption deadline:
            # qk(hp) is needed by block 2*hp, V nh=1 by block 6 (pair 3).
            for hp in (2, 3):
                for m in (hp, hp + 6):
                    for ih in range(2):
                        fill_queue.append(
                            lambda m=m, ih=ih: emit_qk_unit(b0, m, ih)
                        )
            for t in range(NJT):
                fill_queue.append(
                    lambda t=t: emit_v_unit(b0, t, 1)
                )
            for hp in (4, 5):
                for m in (hp, hp + 6):
                    for ih in range(2):
                        fill_queue.append(
                            lambda m=m, ih=ih: emit_qk_unit(b0, m, ih)
                        )
            for m in (0, 6, 1, 7):
                for ih in range(2):
                    fill_queue.append(
                        lambda m=m, ih=ih: emit_qk_unit(b1, m, ih)
                    )
            for t in range(NJT):
                fill_queue.append(lambda t=t: emit_v_unit(b1, t, 0))

            pending = None
            for hp in range(NPAIR):
                for ih in range(2):
                    pending = emit_attn_block(b0, hp, ih, pending)

            # drain b0 leftovers, then queue b1's remaining QKV (deadline
            # order) and b0's proj as filler during b1 attention
            while fill_queue:
                fill_queue.pop(0)()
            for hp in (2, 3):
                for m in (hp, hp + 6):
                    for ih in range(2):
                        fill_queue.append(
                            lambda m=m, ih=ih: emit_qk_unit(b1, m, ih)
                        )
            for t in range(NJT):
                fill_queue.append(lambda t=t: emit_v_unit(b1, t, 1))
            for hp in (4, 5):
                for m in (hp, hp + 6):
                    for ih in range(2):
                        fill_queue.append(
                            lambda m=m, ih=ih: emit_qk_unit(b1, m, ih)
                        )
            for t in range(NJT):
                for nh in range(2):
                    fill_queue.append(
                        lambda t=t, nh=nh: emit_proj_unit(b0, t, nh)
                    )

            for hp in range(NPAIR):
                for ih in range(2):
                    pending = emit_attn_block(b1, hp, ih, pending)
            emit_norm(pending)

            # drain remaining filler (b0 proj leftovers), then b1 proj
            while fill_queue:
                fill_queue.pop(0)()
            for t in range(NJT):
                for nh in range(2):
                    emit_proj_unit(b1, t, nh)

    nc.compile()
    return nc


_NC_CACHE = None


def _get_nc():
    global _NC_CACHE
    if _NC_CACHE is None:
        _NC_CACHE = build()
    return _NC_CACHE


def _prep_core_inputs(x_c, w_qkv, b_qkv, w_proj, b_proj):
    """Host-side layout prep for one core. x_c: [2, 1024, 768]."""
    xT = np.ascontiguousarray(x_c.reshape(T, D).T).astype(np.float32)
    bqk = np.ascontiguousarray(b_qkv[: 12 * 128].reshape(12, 128).T)
    import ml_dtypes

    bf = ml_dtypes.bfloat16
    return {
        "xT": np.ascontiguousarray(xT.astype(bf)),
        "wqkv": np.ascontiguousarray(w_qkv.astype(bf)),
        "wproj": np.ascontiguousarray(w_proj.astype(bf)),
        "bqk": bqk.astype(np.float32),
        "bv": np.ascontiguousarray(b_qkv[2 * D :].reshape(1, D).astype(bf)),
        "bproj": np.ascontiguousarray(b_proj.reshape(1, D).astype(bf)),
    }


def kernel(x, w_qkv, b_qkv, w_proj, b_proj):
    x = np.asarray(x, dtype=np.float32)
    w_qkv = np.asarray(w_qkv, dtype=np.float32)
    b_qkv = np.asarray(b_qkv, dtype=np.float32)
    w_proj = np.asarray(w_proj, dtype=np.float32)
    b_proj = np.asarray(b_proj, dtype=np.float32)
    B, N, Dd = x.shape
    assert (B, N, Dd) == (16, 1024, 768)

    nc = _get_nc()
    in_maps = [
        _prep_core_inputs(x[2 * c : 2 * c + 2], w_qkv, b_qkv, w_proj, b_proj)
        for c in range(8)
    ]
    res = run_bass_kernel_spmd(nc, in_maps, core_ids=list(range(8)))
    out = np.empty((B, N, Dd), dtype=np.float32)
    for c in range(8):
        out[2 * c : 2 * c + 2] = res.results[c]["out"].reshape(2, N, Dd)
    kernel.last_results = res
    return out


# revision 30
# speedup vs baseline: 1.0120x; 1.0120x over previous
"""Multi-head attention kernel for Trainium2, data-parallel over 8 NeuronCores.

Problem: B=16, N=1024, D=768, H=12 heads (hd=64), fp32 I/O.
  qkv = x @ w_qkv + b_qkv ; attention ; out = attn_out @ w_proj + b_proj

Sharding: batch data-parallel — core c handles batches [2c, 2c+2); weights
replicated. Inside each core, the two batches are processed sequentially.

Layout strategy (bf16 matmuls at 1 cyc/row; PE held continuously busy so it
ramps to the 2.4 GHz p-state):
  - host pre-transposes x to xT [768, T] so the in-feature contraction has
    features on partitions for both operands.
  - Q^T, K^T computed feature-major [768, N]: lhsT = w_qkv cols, rhs = xT.
    A 128-row feature tile holds a PAIR of heads (2x64) -> scores matmuls
    for the two heads run packed via tile_position rows (K=64).
  - V computed token-major [N, 768], stored bf16 with a ones column appended
    per head (v_ext [128, 65]).
  - scores^T tiles [128 j, 512 q] per head -> one ACT exp op [128, 1024]
    covers both heads of a pair (softmax scale folded into exp's scale).
  - U^T = sum_j exp * v_ext accumulates in PSUM [65, 512]; row 64 is the
    softmax denominator (ones column).
  - normalization: reciprocal_approx_fast on the two [1,512] denominator
    rows straight from PSUM -> dinv2 [2,512]; a rank-2 f32r outer-product
    matmul broadcasts both rows into rb [128,512] PSUM; one tensor_mul
    (staged U x rb) writes the normalized attn output into uT.
  - proj: lhsT = uT tile, rhs = w_proj; + b_proj via broadcast add.
    Output lands token-major [T, 768] == final layout.
  - scores are emitted one key-tile ahead of attn@V so exp latency is
    hidden; QKV of the next batch / proj of the previous batch are
    interleaved into the attention blocks as TensorE filler.
"""

import contextlib
import ctypes
import os
import sys
import types

import numpy as np

# ---------------------------------------------------------------------------
# NTFF profiling shim: bass_utils's trace path imports
# antenv.axon_hooks.get_axon_ntff_profile_hook, which this container's antenv
# lacks. Register a ctypes-based equivalent so BASS_TRACE=1 works. Harmless
# if tracing is never requested.
# ---------------------------------------------------------------------------


def _install_ntff_shim():
    if "antenv.axon_hooks" in sys.modules:
        return
    so_path = "/opt/axon/libaxon_pjrt.so"
    hook = None
    try:
        lib = ctypes.CDLL(so_path)
        if hasattr(lib, "axon_start_nrt_profile"):
            lib.axon_start_nrt_profile.argtypes = [
                ctypes.POINTER(ctypes.c_int64),
                ctypes.c_size_t,
            ]
            lib.axon_start_nrt_profile.restype = ctypes.c_int64
            lib.axon_stop_nrt_profile.argtypes = [ctypes.c_char_p]
            lib.axon_stop_nrt_profile.restype = ctypes.c_int64

            @contextlib.contextmanager
            def _hook(output_dir, device_ids):
                import jax

                jax.devices()
                if device_ids:
                    ids = (ctypes.c_int64 * len(device_ids))(*device_ids)
                    rc = lib.axon_start_nrt_profile(ids, len(device_ids))
                else:
                    rc = lib.axon_start_nrt_profile(None, 0)
                if rc != 0:
                    raise RuntimeError(f"axon_start_nrt_profile rc={rc}")
                try:
                    yield
                finally:
                    n = lib.axon_stop_nrt_profile(str(output_dir).encode())
                    print(f"ntff profile: {n} file(s) in {output_dir}",
                          file=sys.stderr)

            hook = _hook
    except OSError:
        pass
    mod = types.ModuleType("antenv.axon_hooks")
    mod.get_axon_ntff_profile_hook = lambda: hook
    mod.set_axon_ntff_profile_hook = lambda h: None
    sys.modules["antenv.axon_hooks"] = mod


_install_ntff_shim()

import concourse.bacc as bacc  # noqa: E402
import concourse.mybir as mybir  # noqa: E402
import concourse.tile as tile  # noqa: E402
from concourse.bass_utils import run_bass_kernel_spmd  # noqa: E402

F32 = mybir.dt.float32
F32R = mybir.dt.float32r
BF16 = mybir.dt.bfloat16
AF = mybir.ActivationFunctionType

# Problem constants (per core)
NB = 2        # batches per core
TN = 1024     # tokens per batch
T = NB * TN   # tokens per core
D = 768
H = 12
HD = 64
D3 = 3 * D
KT = D // 128          # 6 contraction tiles
NPAIR = H // 2         # 6 head pairs
NJT = TN // 128        # 8 key tiles per batch
SCALE = HD ** -0.5


def build():
    nc = bacc.Bacc(None)
    xT_d = nc.declare_dram_parameter("xT", [D, T], BF16, isOutput=False)
    wqkv_d = nc.declare_dram_parameter("wqkv", [D, D3], BF16, isOutput=False)
    wproj_d = nc.declare_dram_parameter("wproj", [D, D], BF16, isOutput=False)
    bqk_d = nc.declare_dram_parameter("bqk", [128, 12], F32, isOutput=False)
    bv_d = nc.declare_dram_parameter("bv", [1, D], BF16, isOutput=False)
    bproj_d = nc.declare_dram_parameter("bproj", [1, D], BF16, isOutput=False)
    out_d = nc.declare_dram_parameter("out", [T, D], F32, isOutput=True)

    with tile.TileContext(nc) as tc:
        with (
            nc.allow_low_precision(reason="bf16 attention pipeline"),
            tc.tile_pool(name="const", bufs=1) as cpool,
            tc.tile_pool(name="xu", bufs=2) as xupool,
            tc.tile_pool(name="qk", bufs=2) as qkpool,
            tc.tile_pool(name="vsb", bufs=2) as vpool,
            tc.tile_pool(name="esb", bufs=4) as epool,
            tc.tile_pool(name="stg", bufs=3) as spool,
            tc.tile_pool(name="dnv", bufs=3) as dpool,
            tc.tile_pool(name="rbp", bufs=4) as rbpool,
            tc.tile_pool(name="osb", bufs=3) as opool,
            tc.tile_pool(name="psS", bufs=2, space="PSUM") as psS,
            tc.tile_pool(name="psU", bufs=2, space="PSUM") as psU,
            tc.tile_pool(name="psQ", bufs=2, space="PSUM") as psQ,
        ):
            def dma_striped(dst, src, stripes):
                # split one logical transfer into partition stripes — each
                # dma_start lands on its own hardware queue, so this divides
                # the per-queue descriptor serialization
                step = 128 // stripes
                for s in range(stripes):
                    nc.sync.dma_start(
                        dst[s * step : (s + 1) * step], src[s * step : (s + 1) * step]
                    )

            # ---- constants / weights (resident) ----
            # DMA waits have running-threshold semantics: waiting on transfer
            # N waits on every earlier-emitted transfer too. Emit strictly in
            # need order: x/wqkv k-chunks first (first QK group), biases next
            # (first evacuations), wproj last (first use ~40% into the run).
            wqkv = cpool.tile([128, KT, D3], BF16, tag="wqkv")
            wqkv_src = wqkv_d.ap().rearrange("(ko p) n -> p ko n", p=128)


            # --- per-batch contexts -------------------------------------
            class Batch:
                pass

            bats = []
            for b in range(NB):
                B_ = Batch()
                B_.tok0 = b * TN
                B_.qT = qkpool.tile([128, NPAIR, TN], BF16, tag="qT",
                                    name=f"qT{b}")
                B_.kT = qkpool.tile([128, NPAIR, TN], BF16, tag="kT",
                                    name=f"kT{b}")
                B_.vsb = vpool.tile([128, NJT, H, HD + 1], BF16, tag="v",
                                    name=f"v{b}")
                B_.xT = xupool.tile([128, KT, TN], BF16, tag="x",
                                    name=f"xT{b}")
                B_.uT = xupool.tile([128, KT, TN], BF16, tag="u",
                                    name=f"uT{b}")
                bats.append(B_)

            def emit_x_dma(B_, wq=False):
                # ones column of v_ext via memset — as a DMA this fragments
                # into ~12k 2-byte descriptors and stalls startup ~20us
                nc.vector.memset(B_.vsb[:, :, :, HD : HD + 1], 1.0)
                xT_src = xT_d.ap().rearrange("(ko p) n -> p ko n", p=128)[
                    :, :, B_.tok0 : B_.tok0 + TN
                ]
                for k in range(KT):
                    dma_striped(
                        B_.xT[:, k : k + 1, :], xT_src[:, k : k + 1, :], 2
                    )
                    if wq:
                        dma_striped(
                            wqkv[:, k : k + 1, :], wqkv_src[:, k : k + 1, :], 2
                        )

            def emit_qk_unit(B_, m, ih):
                # one [128 feat, 512 tok] Q^T or K^T tile (m<6: Q, else K)
                dst = B_.qT if m < 6 else B_.kT
                hp = m % 6
                ps = psQ.tile([128, 512], F32, tag="ps")
                for k in range(KT):
                    nc.tensor.matmul(
                        ps[:],
                        wqkv[:, k, m * 128 : (m + 1) * 128],
                        B_.xT[:, k, ih * 512 : (ih + 1) * 512],
                        start=(k == 0),
                        stop=(k == KT - 1),
                    )
                nc.vector.tensor_scalar_add(
                    dst[:, hp, ih * 512 : (ih + 1) * 512],
                    ps[:],
                    bqk[:, m : m + 1],
                )

            def emit_v_unit(B_, t, nh):
                # one [128 tok, 384 feat] V tile into the v_ext slots
                ps = psQ.tile([128, 384], F32, tag="ps")
                for k in range(KT):
                    nc.tensor.matmul(
                        ps[:],
                        B_.xT[:, k, t * 128 : (t + 1) * 128],
                        wqkv[:, k, 2 * D + nh * 384 : 2 * D + (nh + 1) * 384],
                        start=(k == 0),
                        stop=(k == KT - 1),
                    )
                nc.vector.tensor_add(
                    B_.vsb[:, t, nh * 6 : (nh + 1) * 6, 0:HD],
                    ps[:],
                    bvb[:, nh * 384 : (nh + 1) * 384],
                )

            def emit_proj_unit(B_, t, nh):
                ps = psQ.tile([128, 384], F32, tag="ps")
                for k in range(KT):
                    nc.tensor.matmul(
                        ps[:],
                        B_.uT[:, k, t * 128 : (t + 1) * 128],
                        wproj[:, k, nh * 384 : (nh + 1) * 384],
                        start=(k == 0),
                        stop=(k == KT - 1),
                    )
                ot = opool.tile([128, 384], F32, tag="o")
                nc.vector.tensor_add(
                    ot[:], ps[:], bprojb[:, nh * 384 : (nh + 1) * 384]
                )
                nc.sync.dma_start(
                    out_d.ap()[
                        B_.tok0 + t * 128 : B_.tok0 + (t + 1) * 128,
                        nh * 384 : (nh + 1) * 384,
                    ],
                    ot[:],
                )

            # ---- filler queue: TensorE work interleaved into attention ---
            fill_queue = []

            def pop_fill(n):
                for _ in range(n):
                    if fill_queue:
                        fill_queue.pop(0)()

            def emit_norm(pending):
                """Broadcast 1/denom (GpSimd, otherwise idle; full-tile dst
                from partition 0 — the only pattern the library op supports)
                and write the normalized pair block into uT."""
                B_, hp, ih, ust, dinv2 = pending
                for h in range(2):
                    rb = rbpool.tile([128, 512], F32, tag="rb",
                                     name=f"rb{h}")
                    nc.gpsimd.partition_broadcast(
                        rb[:], dinv2[0:1, h * 512 : (h + 1) * 512]
                    )
                    nc.vector.tensor_mul(
                        B_.uT[
                            h * 64 : (h + 1) * 64,
                            hp,
                            ih * 512 : (ih + 1) * 512,
                        ],
                        ust[h * 64 : (h + 1) * 64, :],
                        rb[h * 64 : (h + 1) * 64, :],
                    )

            def emit_attn_block(B_, hp, ih, pending):
                """One (head-pair, query-half) attention block: 8 key tiles of
                scores+exp+attnV, scores one jt ahead of attnV."""
                i0 = ih * 512
                pu = [
                    psU.tile([HD + 1, 512], F32, tag="pu", name=f"pu{h}")
                    for h in range(2)
                ]
                prev_e = None

                def attnv(e, jt):
                    for h in range(2):
                        nc.tensor.matmul(
                            pu[h][:],
                            B_.vsb[:, jt, 2 * hp + h, :],
                            e[:, h * 512 : (h + 1) * 512],
                            start=(jt == 0),
                            stop=(jt == NJT - 1),
                        )

                for jt in range(NJT):
                    ps = psS.tile([128, 1024], F32, tag="s")
                    for h in range(2):
                        nc.tensor.matmul(
                            ps[:, h * 512 : (h + 1) * 512],
                            B_.kT[
                                h * 64 : (h + 1) * 64,
                                hp,
                                jt * 128 : (jt + 1) * 128,
                            ],
                            B_.qT[h * 64 : (h + 1) * 64, hp, i0 : i0 + 512],
                        )
                    e = epool.tile([128, 1024], BF16, tag="e")
                    nc.scalar.activation(e[:], ps[:], AF.Exp, scale=SCALE)
                    if prev_e is not None:
                        attnv(prev_e, jt - 1)
                    if jt == 1 and pending is not None:
                        emit_norm(pending)
                        pending = None
                    if jt in (0, 2, 4, 6):
                        pop_fill(1)
                    prev_e = e
                attnv(prev_e, NJT - 1)
                if pending is not None:
                    emit_norm(pending)

                # evacuate: denominator rows to SBUF staging, then one
                # reciprocal over both; data rows -> ust staging. pu slots
                # release after the copies.
                dtmp = dpool.tile([1, 1024], F32, tag="dtmp", bufs=2)
                dinv2 = dpool.tile([1, 1024], F32, tag="dinv", bufs=2)
                ust = spool.tile([128, 512], F32, tag="ust")
                for h in range(2):
                    nc.vector.tensor_copy(
                        dtmp[0:1, h * 512 : (h + 1) * 512],
                        pu[h][HD : HD + 1, :],
                    )
                    nc.vector.tensor_copy(
                        ust[h * 64 : (h + 1) * 64, :], pu[h][0:HD, :]
                    )
                nc.vector.reciprocal_approx_fast(dinv2[0:1, :], dtmp[0:1, :])
                return (B_, hp, ih, ust, dinv2)

            # ================= global schedule =================
            b0, b1 = bats
            emit_x_dma(b0, wq=True)
            bqk = cpool.tile([128, 12], F32, tag="bqk")
            nc.sync.dma_start(bqk[:], bqk_d.ap())
            bv1 = cpool.tile([1, D], BF16, tag="bv1")
            nc.sync.dma_start(bv1[:], bv_d.ap())
            bvb = cpool.tile([128, D], BF16, tag="bvb")
            nc.gpsimd.partition_broadcast(bvb[:], bv1[:])
            bproj1 = cpool.tile([1, D], BF16, tag="bproj1")
            nc.sync.dma_start(bproj1[:], bproj_d.ap())
            bprojb = cpool.tile([128, D], BF16, tag="bprojb")
            nc.gpsimd.partition_broadcast(bprojb[:], bproj1[:])
            # b0 prologue: V (nh=0 covers head pairs 0..2) + Q/K pairs 0,1
            for m in (0, 6):
                for ih in range(2):
                    emit_qk_unit(b0, m, ih)
            for t in range(NJT):
                emit_v_unit(b0, t, 0)
            for m in (1, 7):
                for ih in range(2):
                    emit_qk_unit(b0, m, ih)
            emit_x_dma(b1)
            wproj = cpool.tile([128, KT, D], BF16, tag="wproj")
            wproj_src = wproj_d.ap().rearrange("(ko p) n -> p ko n", p=128)
            for k in range(KT):
                nc.sync.dma_start(
                    wproj[:, k : k + 1, :], wproj_src[:, k : k + 1, :]
                )

            # filler during b0 attention, ordered by consumption deadline:
            # qk(hp) is needed by block 2*hp, V nh=1 by block 6 (pair 3).
            for hp in (2, 3):
                for m in (hp, hp + 6):
                    for ih in range(2):
                        fill_queue.append(
                            lambda m=m, ih=ih: emit_qk_unit(b0, m, ih)
                        )
            for t in range(NJT):
                fill_queue.append(
                    lambda t=t: emit_v_unit(b0, t, 1)
                )
            for hp in (4, 5):
                for m in (hp, hp + 6):
                    for ih in range(2):
                        fill_queue.append(
                            lambda m=m, ih=ih: emit_qk_unit(b0, m, ih)
                        )
            for m in (0, 6, 1, 7):
                for ih in range(2):
                    fill_queue.append(
                        lambda m=m, ih=ih: emit_qk_unit(b1, m, ih)
                    )
            for t in range(NJT):
                fill_queue.append(lambda t=t: emit_v_unit(b1, t, 0))

            pending = None
            for hp in range(NPAIR):
                for ih in range(2):
                    pending = emit_attn_block(b0, hp, ih, pending)

            # drain b0 leftovers, then queue b1's remaining QKV (deadline
            # order) and b0's proj as filler during b1 attention
            while fill_queue:
                fill_queue.pop(0)()
            for hp in (2, 3):
                for m in (hp, hp + 6):
                    for ih in range(2):
                        fill_queue.append(
                            lambda m=m, ih=ih: emit_qk_unit(b1, m, ih)
                        )
            for t in range(NJT):
                fill_queue.append(lambda t=t: emit_v_unit(b1, t, 1))
            for hp in (4, 5):
                for m in (hp, hp + 6):
                    for ih in range(2):
                        fill_queue.append(
                            lambda m=m, ih=ih: emit_qk_unit(b1, m, ih)
                        )
            for t in range(NJT):
                for nh in range(2):
                    fill_queue.append(
                        lambda t=t, nh=nh: emit_proj_unit(b0, t, nh)
                    )

            for hp in range(NPAIR):
                for ih in range(2):
                    pending = emit_attn_block(b1, hp, ih, pending)
            emit_norm(pending)

            # drain remaining filler (b0 proj leftovers), then b1 proj
            while fill_queue:
                fill_queue.pop(0)()
            for t in range(NJT):
                for nh in range(2):
                    emit_proj_unit(b1, t, nh)

    nc.compile()
    return nc


_NC_CACHE = None


def _get_nc():
    global _NC_CACHE
    if _NC_CACHE is None:
        _NC_CACHE = build()
    return _NC_CACHE


def _prep_core_inputs(x_c, w_qkv, b_qkv, w_proj, b_proj):
    """Host-side layout prep for one core. x_c: [2, 1024, 768]."""
    xT = np.ascontiguousarray(x_c.reshape(T, D).T).astype(np.float32)
    bqk = np.ascontiguousarray(b_qkv[: 12 * 128].reshape(12, 128).T)
    import ml_dtypes

    bf = ml_dtypes.bfloat16
    return {
        "xT": np.ascontiguousarray(xT.astype(bf)),
        "wqkv": np.ascontiguousarray(w_qkv.astype(bf)),
        "wproj": np.ascontiguousarray(w_proj.astype(bf)),
        "bqk": bqk.astype(np.float32),
        "bv": np.ascontiguousarray(b_qkv[2 * D :].reshape(1, D).astype(bf)),
        "bproj": np.ascontiguousarray(b_proj.reshape(1, D).astype(bf)),
    }


def kernel(x, w_qkv, b_qkv, w_proj, b_proj):
    x = np.asarray(x, dtype=np.float32)
    w_qkv = np.asarray(w_qkv, dtype=np.float32)
    b_qkv = np.asarray(b_qkv, dtype=np.float32)
    w_proj = np.asarray(w_proj, dtype=np.float32)
    b_proj = np.asarray(b_proj, dtype=np.float32)
    B, N, Dd = x.shape
    assert (B, N, Dd) == (16, 1024, 768)

    nc = _get_nc()
    in_maps = [
        _prep_core_inputs(x[2 * c : 2 * c + 2], w_qkv, b_qkv, w_proj, b_proj)
        for c in range(8)
    ]
    res = run_bass_kernel_spmd(nc, in_maps, core_ids=list(range(8)))
    out = np.empty((B, N, Dd), dtype=np.float32)
    for c in range(8):
        out[2 * c : 2 * c + 2] = res.results[c]["out"].reshape(2, N, Dd)
    kernel.last_results = res
    return out
